# revision 12
# baseline (speedup 1.0000x reference)
"""BERT multi-head self-attention on 8 Trainium2 NeuronCores (v2).

Problem: B=2, S=2048, H=768, NH=12, HD=64 (fp32 reference).

Sharding (hardcoded): core c in 0..7 handles batch b=c//4 and head group
g=c%4 (heads 3g..3g+2).  Each core computes its 3 heads' attention plus the
partial output projection; the host sums the 4 partial outputs per batch
element and adds the (bv @ Wo + bo) constant row.

The per-core program is PE-bound, so the structure keeps PE dense:
  - QKV projections in split-fp8 DoubleRow (x ~= x8+xr, W ~= 64*(w8+wr),
    all fp8e4m3; 3 terms, 256-row contraction at 0.5 cycles/col) -- 25%
    cheaper than bf16 at bf16-grade accuracy.  Zero-QK-bias specialization;
    general biases fall back to a bf16 build.  The global 64x weight scale
    is undone via the exp scale and a host-side /64.
  - variable-width attention blocks: head0/q-half0 runs as two 512-col
    sub-blocks so the first exp fires early; the last head/q-half runs
    768+256 so the output-projection tail shrinks.
  - ~1/4 of the exp tiles run on DVE as a Schraudolph bit-trick
    (i16 = s*scores + bias read back as bf16), relieving the ACT-paced
    stretches; measured end-to-end rel err 9.1e-3 (gate 2e-2).
  - ctx accumulators split into two single-bank psum tiles so the next
    block's PV can start while the previous block's normalize drains.
  - projection work beyond a 3-unit prefix is emitted as deadline-ordered
    fillers inside the early blocks + at block boundaries; output
    projection packed 2-pass (heads 0+1 stacked into one [128,S] ctxn tile
    via a DMA partition shift) and interleaved into later blocks.
  - ACT runs (most of) the exp stream only; DVE the psum evictions;
    normalize broadcast via gpsimd partition_broadcast; PE-p-state warmup
    matmuls cover the initial DMA window; all DMAs on the sync ring,
    need-ordered (the cost model serializes HWDGE).
"""

import os
import sys
import numpy as np

for _p in ("/opt/trn_rl_repo",):
    if _p not in sys.path and os.path.isdir(_p):
        sys.path.append(_p)

import ml_dtypes  # noqa: E402

from concourse import bacc  # noqa: E402
import concourse.mybir as mybir  # noqa: E402
import concourse.tile as tile  # noqa: E402
from concourse.bass_utils import run_bass_kernel_spmd  # noqa: E402

B, S, H = 2, 2048, 768
NH, HD = 12, 64
HPC = 3            # heads per core
NCORES = 8
P = 128
NKB = S // P       # 16 k-blocks
NQB = S // P       # 16 q-blocks
NHC = H // P       # 6 contraction chunks over hidden dim
QH = 1024          # q-half granularity of ctx/normalize
F32 = mybir.dt.float32

CDT = mybir.dt.bfloat16   # compute dtype for matmul operands
NP_CDT = ml_dtypes.bfloat16

GPSIMD_BCAST = True   # use gpsimd partition_broadcast for 1/denom fanout

# attention blocks: (q0, qw, head).  See module docstring.
BLOCKS = [(0, 512, 0), (512, 512, 0), (0, 1024, 1), (0, 1024, 2),
          (1024, 1024, 0), (1024, 1024, 1), (1024, 768, 2), (1792, 256, 2)]

# (block, kb) tiles whose exp runs on DVE via the Schraudolph bit trick
# (i16 = s*scores + bias, bit pattern read back as bf16 ~= exp(scores/8)).
# ACT paces parts of the stream; offloading ~1/4 of the exps to DVE
# shortens it.  Error cost ~+5e-3 on the end-to-end rel metric (9.1e-3
# total vs the 2e-2 gate).
SCHRAUD = {(bi, kb) for bi in (1, 2, 3, 4, 5, 6) for kb in (4, 9, 12, 14)}
SCHRAUD_POOL = set()
WSCALE = 64.0      # global fp8-path weight scale (keeps residuals normal);
                   # undone via the exp scale (q.k picks up WSCALE^2) and a
                   # host-side /WSCALE on the partial outputs (v path).
SCH_S2 = 127.0 * 128.0 - 5.25                 # exponent bias + sigma


def _build_nc(use_mask: bool, fp8proj: bool):
    import contextlib

    nc = bacc.Bacc("TRN2", target_bir_lowering=False)
    AF = mybir.ActivationFunctionType
    F8 = mybir.dt.float8e4
    DR = mybir.MatmulPerfMode.DoubleRow
    NCP = NHC // 2   # DoubleRow contraction chunk-pairs
    exp_scale = 0.125 / (WSCALE * WSCALE) if fp8proj else 0.125
    sch_s1 = exp_scale * 1.4426950408889634 * 128.0

    if fp8proj:
        # split-fp8 QKT/V projection operands (zero-bias specialization):
        # value ~= v8 + vr with both halves in fp8e4m3 (weights carry a
        # global 64x so residuals stay in normal range).  DoubleRow then
        # contracts 256 rows/pass at 0.5 cycles/col.
        x8 = nc.dram_tensor("x8", [H, S], F8, kind="ExternalInput")
        xr = nc.dram_tensor("xr", [H, S], F8, kind="ExternalInput")
        wqk8 = nc.dram_tensor("wqk8", [H, 2 * HPC * HD], F8,
                              kind="ExternalInput")
        wqkr = nc.dram_tensor("wqkr", [H, 2 * HPC * HD], F8,
                              kind="ExternalInput")
        wv8 = nc.dram_tensor("wv8", [H, HPC * HD], F8, kind="ExternalInput")
        wvr = nc.dram_tensor("wvr", [H, HPC * HD], F8, kind="ExternalInput")
    else:
        xt = nc.dram_tensor("xt", [H, S], CDT, kind="ExternalInput")
        # wqk columns ordered [Q0|Q1|K0|K1|Q2|K2] (64 cols each)
        wqk = nc.dram_tensor("wqk", [H, 2 * HPC * HD], CDT,
                             kind="ExternalInput")
        wv = nc.dram_tensor("wv", [H, HPC * HD], CDT, kind="ExternalInput")
        # bqk rows ordered to match wqk columns
        bqk = nc.dram_tensor("bqk", [2 * HPC * HD, 1], F32,
                             kind="ExternalInput")
    wo = nc.dram_tensor("wo", [HPC * HD, H], CDT, kind="ExternalInput")
    if use_mask:
        mv = nc.dram_tensor("mv", [S, 1], F32, kind="ExternalInput")
    out = nc.dram_tensor("out", [S, H], CDT, kind="ExternalOutput")
    if not GPSIMD_BCAST:
        rspill = nc.dram_tensor("rspill", [2 * HPC, QH], F32)

    with tile.TileContext(nc) as tc, contextlib.ExitStack() as ctx, \
            nc.allow_low_precision(reason="bf16 compute pipeline by design"):
        const = ctx.enter_context(tc.tile_pool(name="const", bufs=1))
        xt_pool = ctx.enter_context(tc.tile_pool(name="xt", bufs=1))
        w_pool = ctx.enter_context(tc.tile_pool(name="w", bufs=1))
        qkt_pool = ctx.enter_context(tc.tile_pool(name="qkt", bufs=1))
        v_pool = ctx.enter_context(tc.tile_pool(name="v", bufs=1))
        pt_pool = ctx.enter_context(tc.tile_pool(name="pt", bufs=7))
        ctxu_pool = ctx.enter_context(tc.tile_pool(name="ctxu", bufs=1))
        ctxn_pool = ctx.enter_context(tc.tile_pool(name="ctxn", bufs=1))
        out_sb_pool = ctx.enter_context(tc.tile_pool(name="outsb", bufs=4))
        rbc_pool = ctx.enter_context(tc.tile_pool(name="rbc", bufs=2))

        # ---- input loads ----
        # The cost model serializes every DMA through one HWDGE device at
        # max(500ns, transfer) each, so: few triggers, strictly need-ordered,
        # all on the sync ring.  First QKT matmul fires ~4us in; everything
        # is resident by ~20us.
        wo01_t = w_pool.tile([P, H], CDT, tag="wo01")
        wo2_t = w_pool.tile([HD, H], CDT, tag="wo2")
        if fp8proj:
            # [128, chunk-pair, 2, n] layouts so DoubleRow lhsT/rhs APs are
            # plain sub-tiles
            x8_t = xt_pool.tile([P, NCP, 2, S], F8, tag="x8")
            xr_t = xt_pool.tile([P, NCP, 2, S], F8, tag="xr")
            wqk8_t = w_pool.tile([P, NCP, 2, 2 * HPC * HD], F8, tag="wqk8")
            wqkr_t = w_pool.tile([P, NCP, 2, 2 * HPC * HD], F8, tag="wqkr")
            wv8_t = w_pool.tile([P, NCP, 2, HPC * HD], F8, tag="wv8")
            wvr_t = w_pool.tile([P, NCP, 2, HPC * HD], F8, tag="wvr")

            def _r(dram, n0, n1):
                return dram[:, n0:n1].rearrange(
                    "(cp two p) n -> p cp two n", p=P, two=2)

            def xt_load(q0, q1):
                nc.sync.dma_start(x8_t[:, :, :, q0:q1], _r(x8, q0, q1))
                nc.sync.dma_start(xr_t[:, :, :, q0:q1], _r(xr, q0, q1))

            nc.sync.dma_start(wqk8_t[:, :, :, 0:256], _r(wqk8, 0, 256))
            nc.sync.dma_start(x8_t[:, :, :, 0:512], _r(x8, 0, 512))
            nc.sync.dma_start(xr_t[:, :, :, 0:512], _r(xr, 0, 512))
            nc.sync.dma_start(wqkr_t[:, :, :, 0:256], _r(wqkr, 0, 256))
            nc.sync.dma_start(wv8_t[:], _r(wv8, 0, HPC * HD))
            nc.sync.dma_start(wvr_t[:], _r(wvr, 0, HPC * HD))
            xt_load(512, 1024)
            nc.sync.dma_start(wqk8_t[:, :, :, 256:384], _r(wqk8, 256, 384))
            nc.sync.dma_start(wqkr_t[:, :, :, 256:384], _r(wqkr, 256, 384))
            xt_load(1024, 1536)
            xt_load(1536, 2048)
        else:
            xt_t = xt_pool.tile([P, NHC, S], CDT, tag="xt")
            xt_sb = [xt_t[:, c, :] for c in range(NHC)]
            wqk_t = w_pool.tile([P, NHC, 2 * HPC * HD], CDT, tag="wqk")
            wqk_r = wqk[:].rearrange("(c p) n -> p c n", p=P)
            wqk_sb = [wqk_t[:, c, :] for c in range(NHC)]
            bias_t = const.tile([P, 3], F32, tag="bqk")
            bias_sb = [bias_t[:, m:m + 1] for m in range(3)]
            wv_t = w_pool.tile([P, NHC, HPC * HD], CDT, tag="wv")
            wv_r = wv[:].rearrange("(c p) n -> p c n", p=P)
            wv_sb = [wv_t[:, c, :] for c in range(NHC)]

            def xt_load(q0, q1):
                nc.sync.dma_start(
                    xt_t[:, :, q0:q1],
                    xt[:, q0:q1].rearrange("(c p) n -> p c n", p=P))

            nc.sync.dma_start(wqk_t[:, :, 0:256], wqk_r[:, :, 0:256])
            xt_load(0, 512)
            nc.sync.dma_start(
                bias_t[:], bqk[:].rearrange("(m p) one -> p (m one)", p=P))
            nc.sync.dma_start(wv_t[:], wv_r[:])
            xt_load(512, 1024)
            nc.sync.dma_start(wqk_t[:, :, 256:384], wqk_r[:, :, 256:384])
            xt_load(1024, 1536)
            xt_load(1536, 2048)
        if use_mask:
            mv_t = const.tile([P, NKB], F32, tag="mv")
            nc.sync.dma_start(
                mv_t[:], mv[:].rearrange("(kb p) one -> p (kb one)", p=P))
            mv_sb = [mv_t[:, kb:kb + 1] for kb in range(NKB)]
        # wo: heads 0,1 stacked [128, H]; head 2 [64, H]
        nc.sync.dma_start(wo01_t[:], wo[0:P, :])
        nc.sync.dma_start(wo2_t[:], wo[P:P + HD, :])

        # ---- QKT projection (m-blocks [Q0|Q1], [K0|K1], [Q2|K2]) and V ----
        tq01 = qkt_pool.tile([P, S], CDT, tag="tq01")
        tk01 = qkt_pool.tile([P, S], CDT, tag="tk01")
        tqk2 = qkt_pool.tile([P, S], CDT, tag="tqk2")
        qkt_tiles = [tq01, tk01, tqk2]
        v_sb = [None] * NKB

        def qkt_unit(psum_tile_fn, m, c0, w):
            qs = slice(c0, c0 + w)
            msl = slice(m * P, (m + 1) * P)
            ps = psum_tile_fn([P, w], "qkps")
            if fp8proj:
                # all three split-fp8 terms are scale-1 (weights carry a
                # global x64 undone via the exp scale), so they share one
                # accumulation group
                groups = ((wqk8_t, x8_t), (wqk8_t, xr_t), (wqkr_t, x8_t))
                for g, (wt, xtile) in enumerate(groups):
                    for cp in range(NCP):
                        nc.tensor.matmul(
                            ps[:], wt[:, cp, :, msl], xtile[:, cp, :, qs],
                            start=(g == 0 and cp == 0),
                            stop=(g == 2 and cp == NCP - 1), perf_mode=DR)
                nc.vector.tensor_copy(qkt_tiles[m][:, qs], ps[:])
            else:
                for c in range(NHC):
                    nc.tensor.matmul(
                        ps[:],
                        wqk_sb[c][:, msl],
                        xt_sb[c][:, qs],
                        start=(c == 0), stop=(c == NHC - 1),
                    )
                nc.vector.tensor_scalar_add(
                    qkt_tiles[m][:, qs], ps[:], bias_sb[m][:]
                )

        def v_unit(psum_tile_fn, kb):
            ksl = slice(kb * P, (kb + 1) * P)
            vt = v_pool.tile([P, HPC, HD + 1], CDT, tag=f"v{kb}",
                             name=f"vt{kb}")
            ps = psum_tile_fn([P, HPC * HD], "vps")
            if fp8proj:
                groups = ((x8_t, wv8_t), (xr_t, wv8_t), (x8_t, wvr_t))
                for g, (xtile, wt) in enumerate(groups):
                    for cp in range(NCP):
                        nc.tensor.matmul(
                            ps[:], xtile[:, cp, :, ksl], wt[:, cp, :, :],
                            start=(g == 0 and cp == 0),
                            stop=(g == 2 and cp == NCP - 1), perf_mode=DR)
            else:
                for c in range(NHC):
                    nc.tensor.matmul(
                        ps[:],
                        xt_sb[c][:, ksl],
                        wv_sb[c][:],
                        start=(c == 0), stop=(c == NHC - 1),
                    )
            nc.vector.tensor_copy(
                vt[:, :, 0:HD], ps[:].rearrange("p (h d) -> p h d", h=HPC)
            )
            nc.vector.memset(vt[:, :, HD:HD + 1], 1.0)
            if use_mask:
                nc.vector.tensor_scalar_mul(vt[:], vt[:], mv_sb[kb][:])
            v_sb[kb] = vt

        # K2 lives at rows 64-127 of tqk2; DMA-shift to its own tile.
        tk2 = qkt_pool.tile([HD, S], CDT, tag="tk2")

        def q_ap(h, sl):  # Q_h^T [64, sl] at base partition 0 or 64
            if h == 0:
                return tq01[0:HD, sl]
            if h == 1:
                return tq01[HD:2 * HD, sl]
            return tqk2[0:HD, sl]

        def k_ap(h, sl):  # K_h^T [64, sl], base partition matching q_ap
            if h == 0:
                return tk01[0:HD, sl]
            if h == 1:
                return tk01[HD:2 * HD, sl]
            return tk2[0:HD, sl]

        # ---- attention state ----
        # ctxn: heads 0,1 stacked in one [128,S] tile (h1 arrives via DMA
        # partition shift); head 2 in its own [64,S] tile.
        stack01 = ctxn_pool.tile([P, S], CDT, tag="stack01")
        ctxn1 = ctxn_pool.tile([HD, S], CDT, tag="ctxn1")
        ctxn2 = ctxn_pool.tile([HD, S], CDT, tag="ctxn2")
        ctxu_t = [ctxu_pool.tile([HD, S], F32, tag=f"ctxu{h}", name=f"ctxu{h}")
                  for h in range(HPC)]
        recip_t = [ctxu_pool.tile([1, S], F32, tag=f"recip{h}",
                                  name=f"recip{h}")
                   for h in range(HPC)]

        def pieces(q0, qw):
            """split [q0, q0+qw) into <=512-wide chunks."""
            res = []
            o = 0
            while o < qw:
                w = min(512, qw - o)
                res.append((o, w))
                o += w
            return res

        def normalize(bi, cps):
            q0, qw, h = BLOCKS[bi]
            qsl = slice(q0, q0 + qw)
            # evict ctx + reciprocal per psum half (releases banks early).
            # In the tail (blocks 6,7) ACT is mostly idle: run the ctx copy
            # there so it overlaps the DVE reciprocal.
            for (o, w), cp in zip(pieces(q0, qw), cps):
                sl = slice(q0 + o, q0 + o + w)
                if bi >= 6:
                    nc.scalar.copy(ctxu_t[h][:, sl], cp[0:HD, :])
                else:
                    nc.vector.tensor_copy(ctxu_t[h][:, sl], cp[0:HD, :])
                nc.vector.reciprocal(recip_t[h][:, sl], cp[HD:HD + 1, :])
            if h == 0:
                dst_t, dp = stack01, 0
            elif h == 1:
                dst_t, dp = ctxn1, 0
            else:
                dst_t, dp = ctxn2, 0
            # last block: broadcast+normalize per 128-col half so each tail
            # op unit can fire as soon as its own q-block is normalized
            widths = [qw] if bi != len(BLOCKS) - 1 else [P] * (qw // P)
            o = 0
            for w in widths:
                sl = slice(q0 + o, q0 + o + w)
                rbc = rbc_pool.tile([HD, QH], F32, tag="rbc", name="rbc")
                nc.gpsimd.partition_broadcast(rbc[:, 0:w], recip_t[h][:, sl])
                nc.vector.tensor_mul(
                    dst_t[dp:dp + HD, sl], ctxu_t[h][:, sl], rbc[:, 0:w])
                o += w
            if h == 1:
                # partition-shift head1 ctxn into rows 64:128 of stack01
                nc.sync.dma_start(stack01[HD:2 * HD, qsl], ctxn1[:, qsl])

        def op_unit(psum_tile_fn, qb, split=False):
            qsl = slice(qb * P, (qb + 1) * P)
            ops = psum_tile_fn([P, H], "ops")
            for nsl in (slice(0, 512), slice(512, H)):
                nc.tensor.matmul(
                    ops[:, nsl], stack01[:, qsl], wo01_t[:, nsl],
                    start=True, stop=False,
                )
                nc.tensor.matmul(
                    ops[:, nsl], ctxn2[:, qsl], wo2_t[:, nsl],
                    start=False, stop=True,
                )
            osb = out_sb_pool.tile([P, H], CDT, tag="osb", name="osb")
            if split:
                # final units: evict+store per half on parallel engines so
                # the last DMA launches as early as possible
                nc.scalar.copy(osb[:, 0:384], ops[:, 0:384])
                nc.sync.dma_start(out[qsl, 0:384], osb[:, 0:384])
                nc.vector.tensor_copy(osb[:, 384:H], ops[:, 384:H])
                nc.sync.dma_start(out[qsl, 384:H], osb[:, 384:H])
            else:
                if qb >= 8 and qb % 2 == 0:
                    nc.scalar.copy(osb[:], ops[:])  # ACT idle in the tail
                else:
                    nc.vector.tensor_copy(osb[:], ops[:])
                nc.sync.dma_start(out[qsl, :], osb[:])

        with tc.tile_pool(name="sc_psum", bufs=3, space="PSUM") as sc_psum, \
             tc.tile_pool(name="ctx_psum", bufs=1, space="PSUM") as ctx_psum:
            def sc_tile(shape, name):
                return sc_psum.tile(shape, F32, tag="sc", name=name)

            # PE p-state warmup: the clock needs ~3us of continuous work to
            # reach 2.4GHz; run throwaway matmuls on a memset tile while the
            # first loads land (psum slot borrowed from the idle ctxB tag).
            wu = const.tile([P, 512], CDT, tag="wu")
            nc.vector.memset(wu[:], 0.25)
            wu_ps = ctx_psum.tile([HD + 1, 512], F32, tag="ctxB", name="wups")
            for _ in range(10):
                nc.tensor.matmul(wu_ps[:], wu[:, 0:HD + 1], wu[:],
                                 start=True, stop=True)

            # prefix: Q01 over q 0:512, K01 over k 0:256, V(0)
            qkt_unit(sc_tile, 0, 0, 512)
            qkt_unit(sc_tile, 1, 0, 256)
            v_unit(sc_tile, 0)

            # fillers keyed by the global scores counter gi (1-based):
            # remaining V and QKT units in deadline order, then boundary
            # coverage for later blocks.
            fill_at = {
                1: [("qk", 1, 256, 256), ("v", 1)],
                2: [("v", 2), ("v", 3)],
                3: [("qk", 1, 512, 512)],
                4: [("v", 4)],
                5: [("v", 5)],
                6: [("v", 6)],
                7: [("qk", 1, 1024, 512)],
                8: [("v", 7)],
                9: [("v", 8), ("v", 9)],
                10: [("v", 10)],
                11: [("qk", 1, 1536, 512)],
                12: [("v", 11)],
                13: [("v", 12)],
                14: [("qk", 0, 512, 512)],
                15: [("v", 13), ("v", 14)],
                16: [("v", 15)],
                # Q2K2 units spread through the ACT-paced b0b/b1 stretch
                # (each <= the lag-3 exp buffer, so they don't stall ACT)
                20: [("qk", 2, 0, 512)],
                28: [("qk", 2, 512, 512)],
                32: [("qk", 2, 1024, 512)],
                41: [("qk", 2, 1536, 512)],
                42: [("k2shift",)],
                # Q01 upper half; the gi-64 unit covers the b2->b3 boundary
                52: [("qk", 0, 1024, 512)],
                64: [("qk", 0, 1536, 512)],
            }
            n_fillers = sum(len(v) for v in fill_at.values())

            def run_filler(u):
                if u[0] == "v":
                    v_unit(sc_tile, u[1])
                elif u[0] == "k2shift":
                    nc.sync.dma_start(tk2[:], tqk2[HD:2 * HD, :])
                else:
                    qkt_unit(sc_tile, u[1], u[2], u[3])

            def scores(bi, kb):
                q0, qw, h = BLOCKS[bi]
                ksl = slice(kb * P, (kb + 1) * P)
                sps = sc_psum.tile([P, qw], F32, tag="sc", name="sps")
                for o, w in pieces(q0, qw):
                    nc.tensor.matmul(
                        sps[:, o:o + w],
                        k_ap(h, ksl),
                        q_ap(h, slice(q0 + o, q0 + o + w)),
                        start=True, stop=True,
                    )
                pt = pt_pool.tile([P, qw], CDT, tag="pt", name="pt")
                if (bi, kb) in SCHRAUD:
                    nc.vector.tensor_scalar(
                        out=pt[:].bitcast(mybir.dt.int16), in0=sps[:],
                        scalar1=sch_s1, scalar2=SCH_S2,
                        op0=mybir.AluOpType.mult, op1=mybir.AluOpType.add)
                elif (bi, kb) in SCHRAUD_POOL:
                    nc.gpsimd.tensor_scalar(
                        out=pt[:].bitcast(mybir.dt.int16), in0=sps[:],
                        scalar1=sch_s1, scalar2=SCH_S2,
                        op0=mybir.AluOpType.mult, op1=mybir.AluOpType.add)
                else:
                    nc.scalar.activation(pt[:], sps[:], AF.Exp,
                                         scale=exp_scale)
                return pt

            def pv(bi, kb, pt, cps):
                _, qw, h = BLOCKS[bi]
                for (o, w), cp in zip(pieces(0, qw), cps):
                    nc.tensor.matmul(
                        cp[:],
                        v_sb[kb][:, h, :],
                        pt[:, o:o + w],
                        start=(kb == 0), stop=(kb == NKB - 1),
                    )

            from collections import deque
            cps_of = {}
            pending = deque()
            norm_done = [False] * len(BLOCKS)

            def drain_one():
                pbi, pkb, ppt = pending.popleft()
                pv(pbi, pkb, ppt, cps_of[pbi])
                if pkb == NKB - 1:
                    normalize(pbi, cps_of[pbi])
                    norm_done[pbi] = True

            gi = 0
            fillers_used = 0
            op_emitted = 0
            for bi in range(len(BLOCKS)):
                q0, qw, h = BLOCKS[bi]
                for kb in range(NKB):
                    if kb == 0:
                        tags = ("ctxA", "ctxB")
                        cps_of[bi] = [
                            ctx_psum.tile([HD + 1, w], F32, tag=tags[i],
                                          name=f"cps{bi}_{i}")
                            for i, (o, w) in enumerate(pieces(0, qw))]
                    pending.append((bi, kb, scores(bi, kb)))
                    gi += 1
                    for u in fill_at.get(gi, ()):
                        run_filler(u)
                        fillers_used += 1
                    # interleave q-half-0 op units once its blocks are done
                    if (norm_done[3] and op_emitted < NQB // 2
                            and gi % 4 == 0):
                        op_unit(sc_tile, op_emitted)
                        op_emitted += 1
                    # q 1024:1920 op units overlap the final 128-wide block
                    if (norm_done[6] and op_emitted >= NQB // 2
                            and op_emitted < 14 and gi % 2 == 0):
                        op_unit(sc_tile, 8 + (op_emitted - NQB // 2))
                        op_emitted += 1
                    lag = 2 if (gi <= 16 or gi > 112) else 3
                    while len(pending) > lag:
                        drain_one()
            while pending:
                drain_one()
            assert fillers_used == n_fillers
            # tail: remaining op units; the very last one splits its
            # evict/store so the final DMA launches earlier
            for qb in range(op_emitted, NQB):
                op_unit(sc_tile, qb)

    nc.compile()
    return nc


_NC_CACHE = {}


def _get_nc(use_mask: bool, fp8proj: bool = True):
    key = (use_mask, fp8proj)
    if key not in _NC_CACHE:
        _NC_CACHE[key] = _build_nc(use_mask, fp8proj)
    return _NC_CACHE[key]


NP_F8 = ml_dtypes.float8_e4m3


def _split8(a):
    """coarse/residual fp8 split: a ~= a8 + ar (elementwise)."""
    a8 = a.astype(NP_F8)
    ar = (a - a8.astype(np.float32)).astype(NP_F8)
    return a8, ar


def _shard_inputs(hidden_states, attention_mask, Wq, bq, Wk, bk, Wv, bv, Wo, bo,
                  use_mask, fp8proj):
    """Build the 8 per-core input maps (all host-side numpy)."""
    in_maps = []
    for c in range(NCORES):
        b, g = divmod(c, NCORES // B)
        cols = slice(g * HPC * HD, (g + 1) * HPC * HD)
        # wqk columns ordered [Q0|Q1|K0|K1|Q2|K2] within the group
        wq_g = Wq[:, cols]
        wk_g = Wk[:, cols]
        qk_cols = [wq_g[:, 0:HD], wq_g[:, HD:2 * HD],
                   wk_g[:, 0:HD], wk_g[:, HD:2 * HD],
                   wq_g[:, 2 * HD:3 * HD], wk_g[:, 2 * HD:3 * HD]]
        wqk = np.concatenate(qk_cols, axis=1)
        m = {"wo": np.ascontiguousarray(Wo[cols, :]).astype(NP_CDT)}
        if fp8proj:
            xt = np.ascontiguousarray(hidden_states[b].T).astype(np.float32)
            x8, xr = _split8(xt)
            w8, wr = _split8(wqk.astype(np.float32) * WSCALE)
            v8, vr = _split8(np.ascontiguousarray(
                Wv[:, cols]).astype(np.float32) * WSCALE)
            m.update(x8=x8, xr=xr, wqk8=w8, wqkr=wr, wv8=v8, wvr=vr)
        else:
            bq_g = bq[cols]
            bk_g = bk[cols]
            bqk = np.concatenate([bq_g[0:HD], bq_g[HD:2 * HD],
                                  bk_g[0:HD], bk_g[HD:2 * HD],
                                  bq_g[2 * HD:3 * HD], bk_g[2 * HD:3 * HD]])
            m.update(
                xt=np.ascontiguousarray(hidden_states[b].T).astype(NP_CDT),
                wqk=np.ascontiguousarray(wqk).astype(NP_CDT),
                wv=np.ascontiguousarray(Wv[:, cols]).astype(NP_CDT),
                bqk=bqk.astype(np.float32).reshape(-1, 1),
            )
        if use_mask:
            mvec = np.exp(-10000.0 * (1.0 - attention_mask[b].astype(np.float64)))
            m["mv"] = mvec.astype(np.float32).reshape(-1, 1)
        in_maps.append(m)
    return in_maps


def kernel(hidden_states, attention_mask, Wq, bq, Wk, bk, Wv, bv, Wo, bo):
    hidden_states = np.asarray(hidden_states, np.float32)
    attention_mask = np.asarray(attention_mask)
    Wq, bq = np.asarray(Wq, np.float32), np.asarray(bq, np.float32)
    Wk, bk = np.asarray(Wk, np.float32), np.asarray(bk, np.float32)
    Wv, bv = np.asarray(Wv, np.float32), np.asarray(bv, np.float32)
    Wo, bo = np.asarray(Wo, np.float32), np.asarray(bo, np.float32)

    use_mask = not bool(np.all(attention_mask == 1))
    # Q/K biases fold into scores on-device; the fp8 projection path is a
    # zero-bias specialization (bv/bo are always handled on the host).
    fp8proj = bool(np.all(bq == 0.0) and np.all(bk == 0.0))
    nc = _get_nc(use_mask, fp8proj)
    in_maps = _shard_inputs(hidden_states, attention_mask,
                            Wq, bq, Wk, bk, Wv, bv, Wo, bo, use_mask, fp8proj)
    res = run_bass_kernel_spmd(nc, in_maps, core_ids=list(range(NCORES)))

    # unshard: sum the 4 head-group partials per batch; add constant row.
    const_row = (bv.astype(np.float64) @ Wo.astype(np.float64)
                 + bo.astype(np.float64))
    out = np.zeros((B, S, H), np.float64)
    for c in range(NCORES):
        b = c // (NCORES // B)
        out[b] += res.results[c]["out"].astype(np.float64)
    if fp8proj:
        out /= WSCALE   # v path carries the global weight scale
    out += const_row[None, None, :]
    return out.astype(np.float32)


if __name__ == "__main__":
    rng = np.random.default_rng(0)
    inputs = {
        "hidden_states": rng.standard_normal((B, S, H)).astype(np.float32),
        "attention_mask": np.ones((B, S), np.int32),
        "Wq": rng.standard_normal((H, H)).astype(np.float32) * 0.02,
        "bq": np.zeros(H, np.float32),
        "Wk": rng.standard_normal((H, H)).astype(np.float32) * 0.02,
        "bk": np.zeros(H, np.float32),
        "Wv": rng.standard_normal((H, H)).astype(np.float32) * 0.02,
        "bv": np.zeros(H, np.float32),
        "Wo": rng.standard_normal((H, H)).astype(np.float32) * 0.02,
        "bo": np.zeros(H, np.float32),
    }
    out = kernel(**inputs)
    print("out", out.shape, out.dtype)


# revision 13
# speedup vs baseline: 1.0003x; 1.0003x over previous
"""BERT multi-head self-attention on 8 Trainium2 NeuronCores (v2).

Problem: B=2, S=2048, H=768, NH=12, HD=64 (fp32 reference).

Sharding (hardcoded): core c in 0..7 handles batch b=c//4 and head group
g=c%4 (heads 3g..3g+2).  Each core computes its 3 heads' attention plus the
partial output projection; the host sums the 4 partial outputs per batch
element and adds the (bv @ Wo + bo) constant row.

The per-core program is PE-bound, so the structure keeps PE dense:
  - QKV projections in split-fp8 DoubleRow (x ~= x8+xr, W ~= 64*(w8+wr),
    all fp8e4m3; 3 terms, 256-row contraction at 0.5 cycles/col) -- 25%
    cheaper than bf16 at bf16-grade accuracy.  Zero-QK-bias specialization;
    general biases fall back to a bf16 build.  The global 64x weight scale
    is undone via the exp scale and a host-side /64.
  - variable-width attention blocks: head0/q-half0 runs as two 512-col
    sub-blocks so the first exp fires early; the last head/q-half runs
    768+256 so the output-projection tail shrinks.
  - ~1/4 of the exp tiles run on DVE as a Schraudolph bit-trick
    (i16 = s*scores + bias read back as bf16), relieving the ACT-paced
    stretches; measured end-to-end rel err 9.1e-3 (gate 2e-2).
  - ctx accumulators split into two single-bank psum tiles so the next
    block's PV can start while the previous block's normalize drains.
  - projection work beyond a 3-unit prefix is emitted as deadline-ordered
    fillers inside the early blocks + at block boundaries; output
    projection packed 2-pass (heads 0+1 stacked into one [128,S] ctxn tile
    via a DMA partition shift) and interleaved into later blocks.
  - ACT runs (most of) the exp stream only; DVE the psum evictions;
    normalize broadcast via gpsimd partition_broadcast; PE-p-state warmup
    matmuls cover the initial DMA window; all DMAs on the sync ring,
    need-ordered (the cost model serializes HWDGE).
"""

import os
import sys
import numpy as np

for _p in ("/opt/trn_rl_repo",):
    if _p not in sys.path and os.path.isdir(_p):
        sys.path.append(_p)

import ml_dtypes  # noqa: E402

from concourse import bacc  # noqa: E402
import concourse.mybir as mybir  # noqa: E402
import concourse.tile as tile  # noqa: E402
from concourse.bass_utils import run_bass_kernel_spmd  # noqa: E402

B, S, H = 2, 2048, 768
NH, HD = 12, 64
HPC = 3            # heads per core
NCORES = 8
P = 128
NKB = S // P       # 16 k-blocks
NQB = S // P       # 16 q-blocks
NHC = H // P       # 6 contraction chunks over hidden dim
QH = 1024          # q-half granularity of ctx/normalize
F32 = mybir.dt.float32

CDT = mybir.dt.bfloat16   # compute dtype for matmul operands
NP_CDT = ml_dtypes.bfloat16

GPSIMD_BCAST = True   # use gpsimd partition_broadcast for 1/denom fanout

# attention blocks: (q0, qw, head).  See module docstring.
BLOCKS = [(0, 512, 0), (512, 512, 0), (0, 1024, 1), (0, 1024, 2),
          (1024, 1024, 0), (1024, 1024, 1), (1024, 768, 2), (1792, 256, 2)]

# (block, kb) tiles whose exp runs on DVE via the Schraudolph bit trick
# (i16 = s*scores + bias, bit pattern read back as bf16 ~= exp(scores/8)).
# ACT paces parts of the stream; offloading ~1/4 of the exps to DVE
# shortens it.  Error cost ~+5e-3 on the end-to-end rel metric (9.1e-3
# total vs the 2e-2 gate).
SCHRAUD = {(bi, kb) for bi in (2, 3, 4, 5, 6) for kb in (4, 9, 12, 14)}
SCHRAUD_POOL = set()
WSCALE = 64.0      # global fp8-path weight scale (keeps residuals normal);
                   # undone via the exp scale (q.k picks up WSCALE^2) and a
                   # host-side /WSCALE on the partial outputs (v path).
SCH_S2 = 127.0 * 128.0 - 5.25                 # exponent bias + sigma


def _build_nc(use_mask: bool, fp8proj: bool):
    import contextlib

    nc = bacc.Bacc("TRN2", target_bir_lowering=False)
    AF = mybir.ActivationFunctionType
    F8 = mybir.dt.float8e4
    DR = mybir.MatmulPerfMode.DoubleRow
    NCP = NHC // 2   # DoubleRow contraction chunk-pairs
    exp_scale = 0.125 / (WSCALE * WSCALE) if fp8proj else 0.125
    sch_s1 = exp_scale * 1.4426950408889634 * 128.0

    if fp8proj:
        # split-fp8 QKT/V projection operands (zero-bias specialization):
        # value ~= v8 + vr with both halves in fp8e4m3 (weights carry a
        # global 64x so residuals stay in normal range).  DoubleRow then
        # contracts 256 rows/pass at 0.5 cycles/col.
        x8 = nc.dram_tensor("x8", [H, S], F8, kind="ExternalInput")
        xr = nc.dram_tensor("xr", [H, S], F8, kind="ExternalInput")
        wqk8 = nc.dram_tensor("wqk8", [H, 2 * HPC * HD], F8,
                              kind="ExternalInput")
        wqkr = nc.dram_tensor("wqkr", [H, 2 * HPC * HD], F8,
                              kind="ExternalInput")
        wv8 = nc.dram_tensor("wv8", [H, HPC * HD], F8, kind="ExternalInput")
        wvr = nc.dram_tensor("wvr", [H, HPC * HD], F8, kind="ExternalInput")
    else:
        xt = nc.dram_tensor("xt", [H, S], CDT, kind="ExternalInput")
        # wqk columns ordered [Q0|Q1|K0|K1|Q2|K2] (64 cols each)
        wqk = nc.dram_tensor("wqk", [H, 2 * HPC * HD], CDT,
                             kind="ExternalInput")
        wv = nc.dram_tensor("wv", [H, HPC * HD], CDT, kind="ExternalInput")
        # bqk rows ordered to match wqk columns
        bqk = nc.dram_tensor("bqk", [2 * HPC * HD, 1], F32,
                             kind="ExternalInput")
    wo = nc.dram_tensor("wo", [HPC * HD, H], CDT, kind="ExternalInput")
    if use_mask:
        mv = nc.dram_tensor("mv", [S, 1], F32, kind="ExternalInput")
    out = nc.dram_tensor("out", [S, H], CDT, kind="ExternalOutput")
    if not GPSIMD_BCAST:
        rspill = nc.dram_tensor("rspill", [2 * HPC, QH], F32)

    with tile.TileContext(nc) as tc, contextlib.ExitStack() as ctx, \
            nc.allow_low_precision(reason="bf16 compute pipeline by design"):
        const = ctx.enter_context(tc.tile_pool(name="const", bufs=1))
        xt_pool = ctx.enter_context(tc.tile_pool(name="xt", bufs=1))
        w_pool = ctx.enter_context(tc.tile_pool(name="w", bufs=1))
        qkt_pool = ctx.enter_context(tc.tile_pool(name="qkt", bufs=1))
        v_pool = ctx.enter_context(tc.tile_pool(name="v", bufs=1))
        pt_pool = ctx.enter_context(tc.tile_pool(name="pt", bufs=7))
        ctxu_pool = ctx.enter_context(tc.tile_pool(name="ctxu", bufs=1))
        ctxn_pool = ctx.enter_context(tc.tile_pool(name="ctxn", bufs=1))
        out_sb_pool = ctx.enter_context(tc.tile_pool(name="outsb", bufs=4))
        rbc_pool = ctx.enter_context(tc.tile_pool(name="rbc", bufs=2))

        # ---- input loads ----
        # The cost model serializes every DMA through one HWDGE device at
        # max(500ns, transfer) each, so: few triggers, strictly need-ordered,
        # all on the sync ring.  First QKT matmul fires ~4us in; everything
        # is resident by ~20us.
        wo01_t = w_pool.tile([P, H], CDT, tag="wo01")
        wo2_t = w_pool.tile([HD, H], CDT, tag="wo2")
        if fp8proj:
            # [128, chunk-pair, 2, n] layouts so DoubleRow lhsT/rhs APs are
            # plain sub-tiles
            x8_t = xt_pool.tile([P, NCP, 2, S], F8, tag="x8")
            xr_t = xt_pool.tile([P, NCP, 2, S], F8, tag="xr")
            wqk8_t = w_pool.tile([P, NCP, 2, 2 * HPC * HD], F8, tag="wqk8")
            wqkr_t = w_pool.tile([P, NCP, 2, 2 * HPC * HD], F8, tag="wqkr")
            wv8_t = w_pool.tile([P, NCP, 2, HPC * HD], F8, tag="wv8")
            wvr_t = w_pool.tile([P, NCP, 2, HPC * HD], F8, tag="wvr")

            def _r(dram, n0, n1):
                return dram[:, n0:n1].rearrange(
                    "(cp two p) n -> p cp two n", p=P, two=2)

            def xt_load(q0, q1):
                nc.sync.dma_start(x8_t[:, :, :, q0:q1], _r(x8, q0, q1))
                nc.sync.dma_start(xr_t[:, :, :, q0:q1], _r(xr, q0, q1))

            nc.sync.dma_start(wqk8_t[:, :, :, 0:256], _r(wqk8, 0, 256))
            nc.sync.dma_start(x8_t[:, :, :, 0:512], _r(x8, 0, 512))
            nc.sync.dma_start(xr_t[:, :, :, 0:512], _r(xr, 0, 512))
            nc.sync.dma_start(wqkr_t[:, :, :, 0:256], _r(wqkr, 0, 256))
            nc.sync.dma_start(wv8_t[:], _r(wv8, 0, HPC * HD))
            nc.sync.dma_start(wvr_t[:], _r(wvr, 0, HPC * HD))
            xt_load(512, 1024)
            nc.sync.dma_start(wqk8_t[:, :, :, 256:384], _r(wqk8, 256, 384))
            nc.sync.dma_start(wqkr_t[:, :, :, 256:384], _r(wqkr, 256, 384))
            xt_load(1024, 1536)
            xt_load(1536, 2048)
        else:
            xt_t = xt_pool.tile([P, NHC, S], CDT, tag="xt")
            xt_sb = [xt_t[:, c, :] for c in range(NHC)]
            wqk_t = w_pool.tile([P, NHC, 2 * HPC * HD], CDT, tag="wqk")
            wqk_r = wqk[:].rearrange("(c p) n -> p c n", p=P)
            wqk_sb = [wqk_t[:, c, :] for c in range(NHC)]
            bias_t = const.tile([P, 3], F32, tag="bqk")
            bias_sb = [bias_t[:, m:m + 1] for m in range(3)]
            wv_t = w_pool.tile([P, NHC, HPC * HD], CDT, tag="wv")
            wv_r = wv[:].rearrange("(c p) n -> p c n", p=P)
            wv_sb = [wv_t[:, c, :] for c in range(NHC)]

            def xt_load(q0, q1):
                nc.sync.dma_start(
                    xt_t[:, :, q0:q1],
                    xt[:, q0:q1].rearrange("(c p) n -> p c n", p=P))

            nc.sync.dma_start(wqk_t[:, :, 0:256], wqk_r[:, :, 0:256])
            xt_load(0, 512)
            nc.sync.dma_start(
                bias_t[:], bqk[:].rearrange("(m p) one -> p (m one)", p=P))
            nc.sync.dma_start(wv_t[:], wv_r[:])
            xt_load(512, 1024)
            nc.sync.dma_start(wqk_t[:, :, 256:384], wqk_r[:, :, 256:384])
            xt_load(1024, 1536)
            xt_load(1536, 2048)
        if use_mask:
            mv_t = const.tile([P, NKB], F32, tag="mv")
            nc.sync.dma_start(
                mv_t[:], mv[:].rearrange("(kb p) one -> p (kb one)", p=P))
            mv_sb = [mv_t[:, kb:kb + 1] for kb in range(NKB)]
        # wo: heads 0,1 stacked [128, H]; head 2 [64, H]
        nc.sync.dma_start(wo01_t[:], wo[0:P, :])
        nc.sync.dma_start(wo2_t[:], wo[P:P + HD, :])

        # ---- QKT projection (m-blocks [Q0|Q1], [K0|K1], [Q2|K2]) and V ----
        tq01 = qkt_pool.tile([P, S], CDT, tag="tq01")
        tk01 = qkt_pool.tile([P, S], CDT, tag="tk01")
        tqk2 = qkt_pool.tile([P, S], CDT, tag="tqk2")
        qkt_tiles = [tq01, tk01, tqk2]
        v_sb = [None] * NKB

        def qkt_unit(psum_tile_fn, m, c0, w):
            qs = slice(c0, c0 + w)
            msl = slice(m * P, (m + 1) * P)
            ps = psum_tile_fn([P, w], "qkps")
            if fp8proj:
                # all three split-fp8 terms are scale-1 (weights carry a
                # global x64 undone via the exp scale), so they share one
                # accumulation group
                groups = ((wqk8_t, x8_t), (wqk8_t, xr_t), (wqkr_t, x8_t))
                for g, (wt, xtile) in enumerate(groups):
                    for cp in range(NCP):
                        nc.tensor.matmul(
                            ps[:], wt[:, cp, :, msl], xtile[:, cp, :, qs],
                            start=(g == 0 and cp == 0),
                            stop=(g == 2 and cp == NCP - 1), perf_mode=DR)
                nc.vector.tensor_copy(qkt_tiles[m][:, qs], ps[:])
            else:
                for c in range(NHC):
                    nc.tensor.matmul(
                        ps[:],
                        wqk_sb[c][:, msl],
                        xt_sb[c][:, qs],
                        start=(c == 0), stop=(c == NHC - 1),
                    )
                nc.vector.tensor_scalar_add(
                    qkt_tiles[m][:, qs], ps[:], bias_sb[m][:]
                )

        def v_unit(psum_tile_fn, kb):
            ksl = slice(kb * P, (kb + 1) * P)
            vt = v_pool.tile([P, HPC, HD + 1], CDT, tag=f"v{kb}",
                             name=f"vt{kb}")
            ps = psum_tile_fn([P, HPC * HD], "vps")
            if fp8proj:
                groups = ((x8_t, wv8_t), (xr_t, wv8_t), (x8_t, wvr_t))
                for g, (xtile, wt) in enumerate(groups):
                    for cp in range(NCP):
                        nc.tensor.matmul(
                            ps[:], xtile[:, cp, :, ksl], wt[:, cp, :, :],
                            start=(g == 0 and cp == 0),
                            stop=(g == 2 and cp == NCP - 1), perf_mode=DR)
            else:
                for c in range(NHC):
                    nc.tensor.matmul(
                        ps[:],
                        xt_sb[c][:, ksl],
                        wv_sb[c][:],
                        start=(c == 0), stop=(c == NHC - 1),
                    )
            nc.vector.tensor_copy(
                vt[:, :, 0:HD], ps[:].rearrange("p (h d) -> p h d", h=HPC)
            )
            nc.vector.memset(vt[:, :, HD:HD + 1], 1.0)
            if use_mask:
                nc.vector.tensor_scalar_mul(vt[:], vt[:], mv_sb[kb][:])
            v_sb[kb] = vt

        # K2 lives at rows 64-127 of tqk2; DMA-shift to its own tile.
        tk2 = qkt_pool.tile([HD, S], CDT, tag="tk2")

        def q_ap(h, sl):  # Q_h^T [64, sl] at base partition 0 or 64
            if h == 0:
                return tq01[0:HD, sl]
            if h == 1:
                return tq01[HD:2 * HD, sl]
            return tqk2[0:HD, sl]

        def k_ap(h, sl):  # K_h^T [64, sl], base partition matching q_ap
            if h == 0:
                return tk01[0:HD, sl]
            if h == 1:
                return tk01[HD:2 * HD, sl]
            return tk2[0:HD, sl]

        # ---- attention state ----
        # ctxn: heads 0,1 stacked in one [128,S] tile (h1 arrives via DMA
        # partition shift); head 2 in its own [64,S] tile.
        stack01 = ctxn_pool.tile([P, S], CDT, tag="stack01")
        ctxn1 = ctxn_pool.tile([HD, S], CDT, tag="ctxn1")
        ctxn2 = ctxn_pool.tile([HD, S], CDT, tag="ctxn2")
        ctxu_t = [ctxu_pool.tile([HD, S], F32, tag=f"ctxu{h}", name=f"ctxu{h}")
                  for h in range(HPC)]
        recip_t = [ctxu_pool.tile([1, S], F32, tag=f"recip{h}",
                                  name=f"recip{h}")
                   for h in range(HPC)]

        def pieces(q0, qw):
            """split [q0, q0+qw) into <=512-wide chunks."""
            res = []
            o = 0
            while o < qw:
                w = min(512, qw - o)
                res.append((o, w))
                o += w
            return res

        def normalize(bi, cps):
            q0, qw, h = BLOCKS[bi]
            qsl = slice(q0, q0 + qw)
            # evict ctx + reciprocal per psum half (releases banks early).
            # In the tail (blocks 6,7) ACT is mostly idle: run the ctx copy
            # there so it overlaps the DVE reciprocal.
            for (o, w), cp in zip(pieces(q0, qw), cps):
                sl = slice(q0 + o, q0 + o + w)
                if bi >= 6:
                    nc.scalar.copy(ctxu_t[h][:, sl], cp[0:HD, :])
                else:
                    nc.vector.tensor_copy(ctxu_t[h][:, sl], cp[0:HD, :])
                nc.vector.reciprocal(recip_t[h][:, sl], cp[HD:HD + 1, :])
            if h == 0:
                dst_t, dp = stack01, 0
            elif h == 1:
                dst_t, dp = ctxn1, 0
            else:
                dst_t, dp = ctxn2, 0
            # last block: broadcast+normalize per 128-col half so each tail
            # op unit can fire as soon as its own q-block is normalized
            widths = [qw] if bi != len(BLOCKS) - 1 else [P] * (qw // P)
            o = 0
            for w in widths:
                sl = slice(q0 + o, q0 + o + w)
                rbc = rbc_pool.tile([HD, QH], F32, tag="rbc", name="rbc")
                nc.gpsimd.partition_broadcast(rbc[:, 0:w], recip_t[h][:, sl])
                nc.vector.tensor_mul(
                    dst_t[dp:dp + HD, sl], ctxu_t[h][:, sl], rbc[:, 0:w])
                o += w
            if h == 1:
                # partition-shift head1 ctxn into rows 64:128 of stack01
                nc.sync.dma_start(stack01[HD:2 * HD, qsl], ctxn1[:, qsl])

        def op_unit(psum_tile_fn, qb, split=False):
            qsl = slice(qb * P, (qb + 1) * P)
            ops = psum_tile_fn([P, H], "ops")
            for nsl in (slice(0, 512), slice(512, H)):
                nc.tensor.matmul(
                    ops[:, nsl], stack01[:, qsl], wo01_t[:, nsl],
                    start=True, stop=False,
                )
                nc.tensor.matmul(
                    ops[:, nsl], ctxn2[:, qsl], wo2_t[:, nsl],
                    start=False, stop=True,
                )
            osb = out_sb_pool.tile([P, H], CDT, tag="osb", name="osb")
            if split:
                # final units: evict+store per half on parallel engines so
                # the last DMA launches as early as possible
                nc.scalar.copy(osb[:, 0:384], ops[:, 0:384])
                nc.sync.dma_start(out[qsl, 0:384], osb[:, 0:384])
                nc.vector.tensor_copy(osb[:, 384:H], ops[:, 384:H])
                nc.sync.dma_start(out[qsl, 384:H], osb[:, 384:H])
            else:
                if qb >= 8 and qb % 2 == 0:
                    nc.scalar.copy(osb[:], ops[:])  # ACT idle in the tail
                else:
                    nc.vector.tensor_copy(osb[:], ops[:])
                nc.sync.dma_start(out[qsl, :], osb[:])

        with tc.tile_pool(name="sc_psum", bufs=3, space="PSUM") as sc_psum, \
             tc.tile_pool(name="ctx_psum", bufs=1, space="PSUM") as ctx_psum:
            def sc_tile(shape, name):
                return sc_psum.tile(shape, F32, tag="sc", name=name)

            # PE p-state warmup: the clock needs ~3us of continuous work to
            # reach 2.4GHz; run throwaway matmuls on a memset tile while the
            # first loads land (psum slot borrowed from the idle ctxB tag).
            wu = const.tile([P, 512], CDT, tag="wu")
            nc.vector.memset(wu[:], 0.25)
            wu_ps = ctx_psum.tile([HD + 1, 512], F32, tag="ctxB", name="wups")
            for _ in range(10):
                nc.tensor.matmul(wu_ps[:], wu[:, 0:HD + 1], wu[:],
                                 start=True, stop=True)

            # prefix: Q01 over q 0:512, K01 over k 0:256, V(0)
            qkt_unit(sc_tile, 0, 0, 512)
            qkt_unit(sc_tile, 1, 0, 256)
            v_unit(sc_tile, 0)

            # fillers keyed by the global scores counter gi (1-based):
            # remaining V and QKT units in deadline order, then boundary
            # coverage for later blocks.
            fill_at = {
                1: [("qk", 1, 256, 256), ("v", 1)],
                2: [("v", 2), ("v", 3)],
                3: [("qk", 1, 512, 512)],
                4: [("v", 4)],
                5: [("v", 5)],
                6: [("v", 6)],
                7: [("qk", 1, 1024, 512)],
                8: [("v", 7)],
                9: [("v", 8), ("v", 9)],
                10: [("v", 10)],
                11: [("qk", 1, 1536, 512)],
                12: [("v", 11)],
                13: [("v", 12)],
                14: [("qk", 0, 512, 512)],
                15: [("v", 13), ("v", 14)],
                16: [("v", 15)],
                # Q2K2 units spread through the ACT-paced b0b/b1 stretch
                # (each <= the lag-3 exp buffer, so they don't stall ACT)
                20: [("qk", 2, 0, 512)],
                28: [("qk", 2, 512, 512)],
                32: [("qk", 2, 1024, 512)],
                41: [("qk", 2, 1536, 512)],
                42: [("k2shift",)],
                # Q01 upper half; the gi-64 unit covers the b2->b3 boundary
                52: [("qk", 0, 1024, 512)],
                64: [("qk", 0, 1536, 512)],
            }
            n_fillers = sum(len(v) for v in fill_at.values())

            def run_filler(u):
                if u[0] == "v":
                    v_unit(sc_tile, u[1])
                elif u[0] == "k2shift":
                    nc.sync.dma_start(tk2[:], tqk2[HD:2 * HD, :])
                else:
                    qkt_unit(sc_tile, u[1], u[2], u[3])

            def scores(bi, kb):
                q0, qw, h = BLOCKS[bi]
                ksl = slice(kb * P, (kb + 1) * P)
                sps = sc_psum.tile([P, qw], F32, tag="sc", name="sps")
                for o, w in pieces(q0, qw):
                    nc.tensor.matmul(
                        sps[:, o:o + w],
                        k_ap(h, ksl),
                        q_ap(h, slice(q0 + o, q0 + o + w)),
                        start=True, stop=True,
                    )
                pt = pt_pool.tile([P, qw], CDT, tag="pt", name="pt")
                if (bi, kb) in SCHRAUD:
                    nc.vector.tensor_scalar(
                        out=pt[:].bitcast(mybir.dt.int16), in0=sps[:],
                        scalar1=sch_s1, scalar2=SCH_S2,
                        op0=mybir.AluOpType.mult, op1=mybir.AluOpType.add)
                elif (bi, kb) in SCHRAUD_POOL:
                    nc.gpsimd.tensor_scalar(
                        out=pt[:].bitcast(mybir.dt.int16), in0=sps[:],
                        scalar1=sch_s1, scalar2=SCH_S2,
                        op0=mybir.AluOpType.mult, op1=mybir.AluOpType.add)
                else:
                    nc.scalar.activation(pt[:], sps[:], AF.Exp,
                                         scale=exp_scale)
                return pt

            def pv(bi, kb, pt, cps):
                _, qw, h = BLOCKS[bi]
                for (o, w), cp in zip(pieces(0, qw), cps):
                    nc.tensor.matmul(
                        cp[:],
                        v_sb[kb][:, h, :],
                        pt[:, o:o + w],
                        start=(kb == 0), stop=(kb == NKB - 1),
                    )

            from collections import deque
            cps_of = {}
            pending = deque()
            norm_done = [False] * len(BLOCKS)

            def drain_one():
                pbi, pkb, ppt = pending.popleft()
                pv(pbi, pkb, ppt, cps_of[pbi])
                if pkb == NKB - 1:
                    normalize(pbi, cps_of[pbi])
                    norm_done[pbi] = True

            gi = 0
            fillers_used = 0
            op_emitted = 0
            for bi in range(len(BLOCKS)):
                q0, qw, h = BLOCKS[bi]
                for kb in range(NKB):
                    if kb == 0:
                        tags = ("ctxA", "ctxB")
                        cps_of[bi] = [
                            ctx_psum.tile([HD + 1, w], F32, tag=tags[i],
                                          name=f"cps{bi}_{i}")
                            for i, (o, w) in enumerate(pieces(0, qw))]
                    pending.append((bi, kb, scores(bi, kb)))
                    gi += 1
                    for u in fill_at.get(gi, ()):
                        run_filler(u)
                        fillers_used += 1
                    # interleave q-half-0 op units once its blocks are done
                    if (norm_done[3] and op_emitted < NQB // 2
                            and gi % 4 == 0):
                        op_unit(sc_tile, op_emitted)
                        op_emitted += 1
                    # q 1024:1920 op units overlap the final 128-wide block
                    if (norm_done[6] and op_emitted >= NQB // 2
                            and op_emitted < 14 and gi % 2 == 0):
                        op_unit(sc_tile, 8 + (op_emitted - NQB // 2))
                        op_emitted += 1
                    lag = 2 if (gi <= 16 or gi > 112) else 3
                    while len(pending) > lag:
                        drain_one()
            while pending:
                drain_one()
            assert fillers_used == n_fillers
            # tail: remaining op units; the very last one splits its
            # evict/store so the final DMA launches earlier
            for qb in range(op_emitted, NQB):
                op_unit(sc_tile, qb)

    nc.compile()
    return nc


_NC_CACHE = {}


def _get_nc(use_mask: bool, fp8proj: bool = True):
    key = (use_mask, fp8proj)
    if key not in _NC_CACHE:
        _NC_CACHE[key] = _build_nc(use_mask, fp8proj)
    return _NC_CACHE[key]


NP_F8 = ml_dtypes.float8_e4m3


def _split8(a):
    """coarse/residual fp8 split: a ~= a8 + ar (elementwise)."""
    a8 = a.astype(NP_F8)
    ar = (a - a8.astype(np.float32)).astype(NP_F8)
    return a8, ar


def _shard_inputs(hidden_states, attention_mask, Wq, bq, Wk, bk, Wv, bv, Wo, bo,
                  use_mask, fp8proj):
    """Build the 8 per-core input maps (all host-side numpy)."""
    in_maps = []
    for c in range(NCORES):
        b, g = divmod(c, NCORES // B)
        cols = slice(g * HPC * HD, (g + 1) * HPC * HD)
        # wqk columns ordered [Q0|Q1|K0|K1|Q2|K2] within the group
        wq_g = Wq[:, cols]
        wk_g = Wk[:, cols]
        qk_cols = [wq_g[:, 0:HD], wq_g[:, HD:2 * HD],
                   wk_g[:, 0:HD], wk_g[:, HD:2 * HD],
                   wq_g[:, 2 * HD:3 * HD], wk_g[:, 2 * HD:3 * HD]]
        wqk = np.concatenate(qk_cols, axis=1)
        m = {"wo": np.ascontiguousarray(Wo[cols, :]).astype(NP_CDT)}
        if fp8proj:
            xt = np.ascontiguousarray(hidden_states[b].T).astype(np.float32)
            x8, xr = _split8(xt)
            w8, wr = _split8(wqk.astype(np.float32) * WSCALE)
            v8, vr = _split8(np.ascontiguousarray(
                Wv[:, cols]).astype(np.float32) * WSCALE)
            m.update(x8=x8, xr=xr, wqk8=w8, wqkr=wr, wv8=v8, wvr=vr)
        else:
            bq_g = bq[cols]
            bk_g = bk[cols]
            bqk = np.concatenate([bq_g[0:HD], bq_g[HD:2 * HD],
                                  bk_g[0:HD], bk_g[HD:2 * HD],
                                  bq_g[2 * HD:3 * HD], bk_g[2 * HD:3 * HD]])
            m.update(
                xt=np.ascontiguousarray(hidden_states[b].T).astype(NP_CDT),
                wqk=np.ascontiguousarray(wqk).astype(NP_CDT),
                wv=np.ascontiguousarray(Wv[:, cols]).astype(NP_CDT),
                bqk=bqk.astype(np.float32).reshape(-1, 1),
            )
        if use_mask:
            mvec = np.exp(-10000.0 * (1.0 - attention_mask[b].astype(np.float64)))
            m["mv"] = mvec.astype(np.float32).reshape(-1, 1)
        in_maps.append(m)
    return in_maps


def kernel(hidden_states, attention_mask, Wq, bq, Wk, bk, Wv, bv, Wo, bo):
    hidden_states = np.asarray(hidden_states, np.float32)
    attention_mask = np.asarray(attention_mask)
    Wq, bq = np.asarray(Wq, np.float32), np.asarray(bq, np.float32)
    Wk, bk = np.asarray(Wk, np.float32), np.asarray(bk, np.float32)
    Wv, bv = np.asarray(Wv, np.float32), np.asarray(bv, np.float32)
    Wo, bo = np.asarray(Wo, np.float32), np.asarray(bo, np.float32)

    use_mask = not bool(np.all(attention_mask == 1))
    # Q/K biases fold into scores on-device; the fp8 projection path is a
    # zero-bias specialization (bv/bo are always handled on the host).
    fp8proj = bool(np.all(bq == 0.0) and np.all(bk == 0.0))
    nc = _get_nc(use_mask, fp8proj)
    in_maps = _shard_inputs(hidden_states, attention_mask,
                            Wq, bq, Wk, bk, Wv, bv, Wo, bo, use_mask, fp8proj)
    res = run_bass_kernel_spmd(nc, in_maps, core_ids=list(range(NCORES)))

    # unshard: sum the 4 head-group partials per batch; add constant row.
    const_row = (bv.astype(np.float64) @ Wo.astype(np.float64)
                 + bo.astype(np.float64))
    out = np.zeros((B, S, H), np.float64)
    for c in range(NCORES):
        b = c // (NCORES // B)
        out[b] += res.results[c]["out"].astype(np.float64)
    if fp8proj:
        out /= WSCALE   # v path carries the global weight scale
    out += const_row[None, None, :]
    return out.astype(np.float32)


if __name__ == "__main__":
    rng = np.random.default_rng(0)
    inputs = {
        "hidden_states": rng.standard_normal((B, S, H)).astype(np.float32),
        "attention_mask": np.ones((B, S), np.int32),
        "Wq": rng.standard_normal((H, H)).astype(np.float32) * 0.02,
        "bq": np.zeros(H, np.float32),
        "Wk": rng.standard_normal((H, H)).astype(np.float32) * 0.02,
        "bk": np.zeros(H, np.float32),
        "Wv": rng.standard_normal((H, H)).astype(np.float32) * 0.02,
        "bv": np.zeros(H, np.float32),
        "Wo": rng.standard_normal((H, H)).astype(np.float32) * 0.02,
        "bo": np.zeros(H, np.float32),
    }
    out = kernel(**inputs)
    print("out", out.shape, out.dtype)


# revision 14
# speedup vs baseline: 1.0007x; 1.0004x over previous
"""BERT multi-head self-attention on 8 Trainium2 NeuronCores (v2).

Problem: B=2, S=2048, H=768, NH=12, HD=64 (fp32 reference).

Sharding (hardcoded): core c in 0..7 handles batch b=c//4 and head group
g=c%4 (heads 3g..3g+2).  Each core computes its 3 heads' attention plus the
partial output projection; the host sums the 4 partial outputs per batch
element and adds the (bv @ Wo + bo) constant row.

The per-core program is PE-bound, so the structure keeps PE dense:
  - QKV projections in split-fp8 DoubleRow (x ~= x8+xr, W ~= 64*(w8+wr),
    all fp8e4m3; 3 terms, 256-row contraction at 0.5 cycles/col) -- 25%
    cheaper than bf16 at bf16-grade accuracy.  Zero-QK-bias specialization;
    general biases fall back to a bf16 build.  The global 64x weight scale
    is undone via the exp scale and a host-side /64.
  - variable-width attention blocks: head0/q-half0 runs as two 512-col
    sub-blocks so the first exp fires early; the last head/q-half runs
    768+256 so the output-projection tail shrinks.
  - ~1/4 of the exp tiles run on DVE as a Schraudolph bit-trick
    (i16 = s*scores + bias read back as bf16), relieving the ACT-paced
    stretches; measured end-to-end rel err 9.1e-3 (gate 2e-2).
  - ctx accumulators split into two single-bank psum tiles so the next
    block's PV can start while the previous block's normalize drains.
  - projection work beyond a 3-unit prefix is emitted as deadline-ordered
    fillers inside the early blocks + at block boundaries; output
    projection packed 2-pass (heads 0+1 stacked into one [128,S] ctxn tile
    via a DMA partition shift) and interleaved into later blocks.
  - ACT runs (most of) the exp stream only; DVE the psum evictions;
    normalize broadcast via gpsimd partition_broadcast; PE-p-state warmup
    matmuls cover the initial DMA window; all DMAs on the sync ring,
    need-ordered (the cost model serializes HWDGE).
"""

import os
import sys
import numpy as np

for _p in ("/opt/trn_rl_repo",):
    if _p not in sys.path and os.path.isdir(_p):
        sys.path.append(_p)

import ml_dtypes  # noqa: E402

from concourse import bacc  # noqa: E402
import concourse.mybir as mybir  # noqa: E402
import concourse.tile as tile  # noqa: E402
from concourse.bass_utils import run_bass_kernel_spmd  # noqa: E402

B, S, H = 2, 2048, 768
NH, HD = 12, 64
HPC = 3            # heads per core
NCORES = 8
P = 128
NKB = S // P       # 16 k-blocks
NQB = S // P       # 16 q-blocks
NHC = H // P       # 6 contraction chunks over hidden dim
QH = 1024          # q-half granularity of ctx/normalize
F32 = mybir.dt.float32

CDT = mybir.dt.bfloat16   # compute dtype for matmul operands
NP_CDT = ml_dtypes.bfloat16

GPSIMD_BCAST = True   # use gpsimd partition_broadcast for 1/denom fanout

# attention blocks: (q0, qw, head).  See module docstring.
BLOCKS = [(0, 512, 0), (512, 512, 0), (0, 1024, 1), (0, 1024, 2),
          (1024, 1024, 0), (1024, 1024, 1), (1024, 768, 2), (1792, 256, 2)]

# (block, kb) tiles whose exp runs on DVE via the Schraudolph bit trick
# (i16 = s*scores + bias, bit pattern read back as bf16 ~= exp(scores/8)).
# ACT paces parts of the stream; offloading ~1/4 of the exps to DVE
# shortens it.  Error cost ~+5e-3 on the end-to-end rel metric (9.1e-3
# total vs the 2e-2 gate).
SCHRAUD = {(bi, kb) for bi in (2, 3, 4, 5, 6) for kb in (4, 9, 12, 14)}
SCHRAUD_POOL = set()
WSCALE = 64.0      # global fp8-path weight scale (keeps residuals normal);
                   # undone via the exp scale (q.k picks up WSCALE^2) and a
                   # host-side /WSCALE on the partial outputs (v path).
SCH_S2 = 127.0 * 128.0 - 5.25                 # exponent bias + sigma


def _build_nc(use_mask: bool, fp8proj: bool):
    import contextlib

    nc = bacc.Bacc("TRN2", target_bir_lowering=False)
    AF = mybir.ActivationFunctionType
    F8 = mybir.dt.float8e4
    DR = mybir.MatmulPerfMode.DoubleRow
    NCP = NHC // 2   # DoubleRow contraction chunk-pairs
    exp_scale = 0.125 / (WSCALE * WSCALE) if fp8proj else 0.125
    sch_s1 = exp_scale * 1.4426950408889634 * 128.0

    if fp8proj:
        # split-fp8 QKT/V projection operands (zero-bias specialization):
        # value ~= v8 + vr with both halves in fp8e4m3 (weights carry a
        # global 64x so residuals stay in normal range).  DoubleRow then
        # contracts 256 rows/pass at 0.5 cycles/col.
        x8 = nc.dram_tensor("x8", [H, S], F8, kind="ExternalInput")
        xr = nc.dram_tensor("xr", [H, S], F8, kind="ExternalInput")
        wqk8 = nc.dram_tensor("wqk8", [H, 2 * HPC * HD], F8,
                              kind="ExternalInput")
        wqkr = nc.dram_tensor("wqkr", [H, 2 * HPC * HD], F8,
                              kind="ExternalInput")
        wv8 = nc.dram_tensor("wv8", [H, HPC * HD], F8, kind="ExternalInput")
        wvr = nc.dram_tensor("wvr", [H, HPC * HD], F8, kind="ExternalInput")
    else:
        xt = nc.dram_tensor("xt", [H, S], CDT, kind="ExternalInput")
        # wqk columns ordered [Q0|Q1|K0|K1|Q2|K2] (64 cols each)
        wqk = nc.dram_tensor("wqk", [H, 2 * HPC * HD], CDT,
                             kind="ExternalInput")
        wv = nc.dram_tensor("wv", [H, HPC * HD], CDT, kind="ExternalInput")
        # bqk rows ordered to match wqk columns
        bqk = nc.dram_tensor("bqk", [2 * HPC * HD, 1], F32,
                             kind="ExternalInput")
    wo = nc.dram_tensor("wo", [HPC * HD, H], CDT, kind="ExternalInput")
    if use_mask:
        mv = nc.dram_tensor("mv", [S, 1], F32, kind="ExternalInput")
    out = nc.dram_tensor("out", [S, H], CDT, kind="ExternalOutput")
    if not GPSIMD_BCAST:
        rspill = nc.dram_tensor("rspill", [2 * HPC, QH], F32)

    with tile.TileContext(nc) as tc, contextlib.ExitStack() as ctx, \
            nc.allow_low_precision(reason="bf16 compute pipeline by design"):
        const = ctx.enter_context(tc.tile_pool(name="const", bufs=1))
        xt_pool = ctx.enter_context(tc.tile_pool(name="xt", bufs=1))
        w_pool = ctx.enter_context(tc.tile_pool(name="w", bufs=1))
        qkt_pool = ctx.enter_context(tc.tile_pool(name="qkt", bufs=1))
        v_pool = ctx.enter_context(tc.tile_pool(name="v", bufs=1))
        pt_pool = ctx.enter_context(tc.tile_pool(name="pt", bufs=7))
        ctxu_pool = ctx.enter_context(tc.tile_pool(name="ctxu", bufs=1))
        ctxn_pool = ctx.enter_context(tc.tile_pool(name="ctxn", bufs=1))
        out_sb_pool = ctx.enter_context(tc.tile_pool(name="outsb", bufs=4))
        rbc_pool = ctx.enter_context(tc.tile_pool(name="rbc", bufs=4))

        # ---- input loads ----
        # The cost model serializes every DMA through one HWDGE device at
        # max(500ns, transfer) each, so: few triggers, strictly need-ordered,
        # all on the sync ring.  First QKT matmul fires ~4us in; everything
        # is resident by ~20us.
        wo01_t = w_pool.tile([P, H], CDT, tag="wo01")
        wo2_t = w_pool.tile([HD, H], CDT, tag="wo2")
        if fp8proj:
            # [128, chunk-pair, 2, n] layouts so DoubleRow lhsT/rhs APs are
            # plain sub-tiles
            x8_t = xt_pool.tile([P, NCP, 2, S], F8, tag="x8")
            xr_t = xt_pool.tile([P, NCP, 2, S], F8, tag="xr")
            wqk8_t = w_pool.tile([P, NCP, 2, 2 * HPC * HD], F8, tag="wqk8")
            wqkr_t = w_pool.tile([P, NCP, 2, 2 * HPC * HD], F8, tag="wqkr")
            wv8_t = w_pool.tile([P, NCP, 2, HPC * HD], F8, tag="wv8")
            wvr_t = w_pool.tile([P, NCP, 2, HPC * HD], F8, tag="wvr")

            def _r(dram, n0, n1):
                return dram[:, n0:n1].rearrange(
                    "(cp two p) n -> p cp two n", p=P, two=2)

            def xt_load(q0, q1):
                nc.sync.dma_start(x8_t[:, :, :, q0:q1], _r(x8, q0, q1))
                nc.sync.dma_start(xr_t[:, :, :, q0:q1], _r(xr, q0, q1))

            nc.sync.dma_start(wqk8_t[:, :, :, 0:256], _r(wqk8, 0, 256))
            nc.sync.dma_start(x8_t[:, :, :, 0:512], _r(x8, 0, 512))
            nc.sync.dma_start(xr_t[:, :, :, 0:512], _r(xr, 0, 512))
            nc.sync.dma_start(wqkr_t[:, :, :, 0:256], _r(wqkr, 0, 256))
            nc.sync.dma_start(wv8_t[:], _r(wv8, 0, HPC * HD))
            nc.sync.dma_start(wvr_t[:], _r(wvr, 0, HPC * HD))
            xt_load(512, 1024)
            nc.sync.dma_start(wqk8_t[:, :, :, 256:384], _r(wqk8, 256, 384))
            nc.sync.dma_start(wqkr_t[:, :, :, 256:384], _r(wqkr, 256, 384))
            xt_load(1024, 1536)
            xt_load(1536, 2048)
        else:
            xt_t = xt_pool.tile([P, NHC, S], CDT, tag="xt")
            xt_sb = [xt_t[:, c, :] for c in range(NHC)]
            wqk_t = w_pool.tile([P, NHC, 2 * HPC * HD], CDT, tag="wqk")
            wqk_r = wqk[:].rearrange("(c p) n -> p c n", p=P)
            wqk_sb = [wqk_t[:, c, :] for c in range(NHC)]
            bias_t = const.tile([P, 3], F32, tag="bqk")
            bias_sb = [bias_t[:, m:m + 1] for m in range(3)]
            wv_t = w_pool.tile([P, NHC, HPC * HD], CDT, tag="wv")
            wv_r = wv[:].rearrange("(c p) n -> p c n", p=P)
            wv_sb = [wv_t[:, c, :] for c in range(NHC)]

            def xt_load(q0, q1):
                nc.sync.dma_start(
                    xt_t[:, :, q0:q1],
                    xt[:, q0:q1].rearrange("(c p) n -> p c n", p=P))

            nc.sync.dma_start(wqk_t[:, :, 0:256], wqk_r[:, :, 0:256])
            xt_load(0, 512)
            nc.sync.dma_start(
                bias_t[:], bqk[:].rearrange("(m p) one -> p (m one)", p=P))
            nc.sync.dma_start(wv_t[:], wv_r[:])
            xt_load(512, 1024)
            nc.sync.dma_start(wqk_t[:, :, 256:384], wqk_r[:, :, 256:384])
            xt_load(1024, 1536)
            xt_load(1536, 2048)
        if use_mask:
            mv_t = const.tile([P, NKB], F32, tag="mv")
            nc.sync.dma_start(
                mv_t[:], mv[:].rearrange("(kb p) one -> p (kb one)", p=P))
            mv_sb = [mv_t[:, kb:kb + 1] for kb in range(NKB)]
        # wo: heads 0,1 stacked [128, H]; head 2 [64, H]
        nc.sync.dma_start(wo01_t[:], wo[0:P, :])
        nc.sync.dma_start(wo2_t[:], wo[P:P + HD, :])

        # ---- QKT projection (m-blocks [Q0|Q1], [K0|K1], [Q2|K2]) and V ----
        tq01 = qkt_pool.tile([P, S], CDT, tag="tq01")
        tk01 = qkt_pool.tile([P, S], CDT, tag="tk01")
        tqk2 = qkt_pool.tile([P, S], CDT, tag="tqk2")
        qkt_tiles = [tq01, tk01, tqk2]
        v_sb = [None] * NKB

        def qkt_unit(psum_tile_fn, m, c0, w):
            qs = slice(c0, c0 + w)
            msl = slice(m * P, (m + 1) * P)
            ps = psum_tile_fn([P, w], "qkps")
            if fp8proj:
                # all three split-fp8 terms are scale-1 (weights carry a
                # global x64 undone via the exp scale), so they share one
                # accumulation group
                groups = ((wqk8_t, x8_t), (wqk8_t, xr_t), (wqkr_t, x8_t))
                for g, (wt, xtile) in enumerate(groups):
                    for cp in range(NCP):
                        nc.tensor.matmul(
                            ps[:], wt[:, cp, :, msl], xtile[:, cp, :, qs],
                            start=(g == 0 and cp == 0),
                            stop=(g == 2 and cp == NCP - 1), perf_mode=DR)
                nc.vector.tensor_copy(qkt_tiles[m][:, qs], ps[:])
            else:
                for c in range(NHC):
                    nc.tensor.matmul(
                        ps[:],
                        wqk_sb[c][:, msl],
                        xt_sb[c][:, qs],
                        start=(c == 0), stop=(c == NHC - 1),
                    )
                nc.vector.tensor_scalar_add(
                    qkt_tiles[m][:, qs], ps[:], bias_sb[m][:]
                )

        def v_unit(psum_tile_fn, kb):
            ksl = slice(kb * P, (kb + 1) * P)
            vt = v_pool.tile([P, HPC, HD + 1], CDT, tag=f"v{kb}",
                             name=f"vt{kb}")
            ps = psum_tile_fn([P, HPC * HD], "vps")
            if fp8proj:
                groups = ((x8_t, wv8_t), (xr_t, wv8_t), (x8_t, wvr_t))
                for g, (xtile, wt) in enumerate(groups):
                    for cp in range(NCP):
                        nc.tensor.matmul(
                            ps[:], xtile[:, cp, :, ksl], wt[:, cp, :, :],
                            start=(g == 0 and cp == 0),
                            stop=(g == 2 and cp == NCP - 1), perf_mode=DR)
            else:
                for c in range(NHC):
                    nc.tensor.matmul(
                        ps[:],
                        xt_sb[c][:, ksl],
                        wv_sb[c][:],
                        start=(c == 0), stop=(c == NHC - 1),
                    )
            nc.vector.tensor_copy(
                vt[:, :, 0:HD], ps[:].rearrange("p (h d) -> p h d", h=HPC)
            )
            nc.vector.memset(vt[:, :, HD:HD + 1], 1.0)
            if use_mask:
                nc.vector.tensor_scalar_mul(vt[:], vt[:], mv_sb[kb][:])
            v_sb[kb] = vt

        # K2 lives at rows 64-127 of tqk2; DMA-shift to its own tile.
        tk2 = qkt_pool.tile([HD, S], CDT, tag="tk2")

        def q_ap(h, sl):  # Q_h^T [64, sl] at base partition 0 or 64
            if h == 0:
                return tq01[0:HD, sl]
            if h == 1:
                return tq01[HD:2 * HD, sl]
            return tqk2[0:HD, sl]

        def k_ap(h, sl):  # K_h^T [64, sl], base partition matching q_ap
            if h == 0:
                return tk01[0:HD, sl]
            if h == 1:
                return tk01[HD:2 * HD, sl]
            return tk2[0:HD, sl]

        # ---- attention state ----
        # ctxn: heads 0,1 stacked in one [128,S] tile (h1 arrives via DMA
        # partition shift); head 2 in its own [64,S] tile.
        stack01 = ctxn_pool.tile([P, S], CDT, tag="stack01")
        ctxn1 = ctxn_pool.tile([HD, S], CDT, tag="ctxn1")
        ctxn2 = ctxn_pool.tile([HD, S], CDT, tag="ctxn2")
        ctxu_t = [ctxu_pool.tile([HD, S], F32, tag=f"ctxu{h}", name=f"ctxu{h}")
                  for h in range(HPC)]
        recip_t = [ctxu_pool.tile([1, S], F32, tag=f"recip{h}",
                                  name=f"recip{h}")
                   for h in range(HPC)]

        def pieces(q0, qw):
            """split [q0, q0+qw) into <=512-wide chunks."""
            res = []
            o = 0
            while o < qw:
                w = min(512, qw - o)
                res.append((o, w))
                o += w
            return res

        def normalize(bi, cps):
            q0, qw, h = BLOCKS[bi]
            qsl = slice(q0, q0 + qw)
            # evict ctx + reciprocal per psum half (releases banks early).
            # In the tail (blocks 6,7) ACT is mostly idle: run the ctx copy
            # there so it overlaps the DVE reciprocal.
            for (o, w), cp in zip(pieces(q0, qw), cps):
                sl = slice(q0 + o, q0 + o + w)
                if bi >= 6:
                    nc.scalar.copy(ctxu_t[h][:, sl], cp[0:HD, :])
                else:
                    nc.vector.tensor_copy(ctxu_t[h][:, sl], cp[0:HD, :])
                nc.vector.reciprocal(recip_t[h][:, sl], cp[HD:HD + 1, :])
            if h == 0:
                dst_t, dp = stack01, 0
            elif h == 1:
                dst_t, dp = ctxn1, 0
            else:
                dst_t, dp = ctxn2, 0
            # last block: broadcast+normalize per 128-col half so each tail
            # op unit can fire as soon as its own q-block is normalized
            widths = [qw] if bi != len(BLOCKS) - 1 else [P] * (qw // P)
            o = 0
            for w in widths:
                sl = slice(q0 + o, q0 + o + w)
                rbc = rbc_pool.tile([HD, QH], F32, tag="rbc", name="rbc")
                nc.gpsimd.partition_broadcast(rbc[:, 0:w], recip_t[h][:, sl])
                nc.vector.tensor_mul(
                    dst_t[dp:dp + HD, sl], ctxu_t[h][:, sl], rbc[:, 0:w])
                o += w
            if h == 1:
                # partition-shift head1 ctxn into rows 64:128 of stack01
                nc.sync.dma_start(stack01[HD:2 * HD, qsl], ctxn1[:, qsl])

        def op_unit(psum_tile_fn, qb, split=False):
            qsl = slice(qb * P, (qb + 1) * P)
            ops = psum_tile_fn([P, H], "ops")
            for nsl in (slice(0, 512), slice(512, H)):
                nc.tensor.matmul(
                    ops[:, nsl], stack01[:, qsl], wo01_t[:, nsl],
                    start=True, stop=False,
                )
                nc.tensor.matmul(
                    ops[:, nsl], ctxn2[:, qsl], wo2_t[:, nsl],
                    start=False, stop=True,
                )
            osb = out_sb_pool.tile([P, H], CDT, tag="osb", name="osb")
            if split:
                # final units: evict+store per half on parallel engines so
                # the last DMA launches as early as possible
                nc.scalar.copy(osb[:, 0:384], ops[:, 0:384])
                nc.sync.dma_start(out[qsl, 0:384], osb[:, 0:384])
                nc.vector.tensor_copy(osb[:, 384:H], ops[:, 384:H])
                nc.sync.dma_start(out[qsl, 384:H], osb[:, 384:H])
            else:
                if qb >= 8 and qb % 2 == 0:
                    nc.scalar.copy(osb[:], ops[:])  # ACT idle in the tail
                else:
                    nc.vector.tensor_copy(osb[:], ops[:])
                nc.sync.dma_start(out[qsl, :], osb[:])

        with tc.tile_pool(name="sc_psum", bufs=3, space="PSUM") as sc_psum, \
             tc.tile_pool(name="ctx_psum", bufs=1, space="PSUM") as ctx_psum:
            def sc_tile(shape, name):
                return sc_psum.tile(shape, F32, tag="sc", name=name)

            # PE p-state warmup: the clock needs ~3us of continuous work to
            # reach 2.4GHz; run throwaway matmuls on a memset tile while the
            # first loads land (psum slot borrowed from the idle ctxB tag).
            wu = const.tile([P, 512], CDT, tag="wu")
            nc.vector.memset(wu[:], 0.25)
            wu_ps = ctx_psum.tile([HD + 1, 512], F32, tag="ctxB", name="wups")
            for _ in range(10):
                nc.tensor.matmul(wu_ps[:], wu[:, 0:HD + 1], wu[:],
                                 start=True, stop=True)

            # prefix: Q01 over q 0:512, K01 over k 0:256, V(0)
            qkt_unit(sc_tile, 0, 0, 512)
            qkt_unit(sc_tile, 1, 0, 256)
            v_unit(sc_tile, 0)

            # fillers keyed by the global scores counter gi (1-based):
            # remaining V and QKT units in deadline order, then boundary
            # coverage for later blocks.
            fill_at = {
                1: [("qk", 1, 256, 256), ("v", 1)],
                2: [("v", 2), ("v", 3)],
                3: [("qk", 1, 512, 512)],
                4: [("v", 4)],
                5: [("v", 5)],
                6: [("v", 6)],
                7: [("qk", 1, 1024, 512)],
                8: [("v", 7)],
                9: [("v", 8), ("v", 9)],
                10: [("v", 10)],
                11: [("qk", 1, 1536, 512)],
                12: [("v", 11)],
                13: [("v", 12)],
                14: [("qk", 0, 512, 512)],
                15: [("v", 13), ("v", 14)],
                16: [("v", 15)],
                # Q2K2 units spread through the ACT-paced b0b/b1 stretch
                # (each <= the lag-3 exp buffer, so they don't stall ACT)
                20: [("qk", 2, 0, 512)],
                28: [("qk", 2, 512, 512)],
                32: [("qk", 2, 1024, 512)],
                41: [("qk", 2, 1536, 512)],
                42: [("k2shift",)],
                # Q01 upper half; the gi-64 unit covers the b2->b3 boundary
                52: [("qk", 0, 1024, 512)],
                64: [("qk", 0, 1536, 512)],
            }
            n_fillers = sum(len(v) for v in fill_at.values())

            def run_filler(u):
                if u[0] == "v":
                    v_unit(sc_tile, u[1])
                elif u[0] == "k2shift":
                    nc.sync.dma_start(tk2[:], tqk2[HD:2 * HD, :])
                else:
                    qkt_unit(sc_tile, u[1], u[2], u[3])

            def scores(bi, kb):
                q0, qw, h = BLOCKS[bi]
                ksl = slice(kb * P, (kb + 1) * P)
                sps = sc_psum.tile([P, qw], F32, tag="sc", name="sps")
                for o, w in pieces(q0, qw):
                    nc.tensor.matmul(
                        sps[:, o:o + w],
                        k_ap(h, ksl),
                        q_ap(h, slice(q0 + o, q0 + o + w)),
                        start=True, stop=True,
                    )
                pt = pt_pool.tile([P, qw], CDT, tag="pt", name="pt")
                if (bi, kb) in SCHRAUD:
                    nc.vector.tensor_scalar(
                        out=pt[:].bitcast(mybir.dt.int16), in0=sps[:],
                        scalar1=sch_s1, scalar2=SCH_S2,
                        op0=mybir.AluOpType.mult, op1=mybir.AluOpType.add)
                elif (bi, kb) in SCHRAUD_POOL:
                    nc.gpsimd.tensor_scalar(
                        out=pt[:].bitcast(mybir.dt.int16), in0=sps[:],
                        scalar1=sch_s1, scalar2=SCH_S2,
                        op0=mybir.AluOpType.mult, op1=mybir.AluOpType.add)
                else:
                    nc.scalar.activation(pt[:], sps[:], AF.Exp,
                                         scale=exp_scale)
                return pt

            def pv(bi, kb, pt, cps):
                _, qw, h = BLOCKS[bi]
                for (o, w), cp in zip(pieces(0, qw), cps):
                    nc.tensor.matmul(
                        cp[:],
                        v_sb[kb][:, h, :],
                        pt[:, o:o + w],
                        start=(kb == 0), stop=(kb == NKB - 1),
                    )

            from collections import deque
            cps_of = {}
            pending = deque()
            norm_done = [False] * len(BLOCKS)

            def drain_one():
                pbi, pkb, ppt = pending.popleft()
                pv(pbi, pkb, ppt, cps_of[pbi])
                if pkb == NKB - 1:
                    normalize(pbi, cps_of[pbi])
                    norm_done[pbi] = True

            gi = 0
            fillers_used = 0
            op_emitted = 0
            for bi in range(len(BLOCKS)):
                q0, qw, h = BLOCKS[bi]
                for kb in range(NKB):
                    if kb == 0:
                        tags = ("ctxA", "ctxB")
                        cps_of[bi] = [
                            ctx_psum.tile([HD + 1, w], F32, tag=tags[i],
                                          name=f"cps{bi}_{i}")
                            for i, (o, w) in enumerate(pieces(0, qw))]
                    pending.append((bi, kb, scores(bi, kb)))
                    gi += 1
                    for u in fill_at.get(gi, ()):
                        run_filler(u)
                        fillers_used += 1
                    # interleave q-half-0 op units once its blocks are done
                    if (norm_done[3] and op_emitted < NQB // 2
                            and gi % 4 == 0):
                        op_unit(sc_tile, op_emitted)
                        op_emitted += 1
                    # q 1024:1920 op units overlap the final 128-wide block
                    if (norm_done[6] and op_emitted >= NQB // 2
                            and op_emitted < 14 and gi % 2 == 0):
                        op_unit(sc_tile, 8 + (op_emitted - NQB // 2))
                        op_emitted += 1
                    lag = 2 if (gi <= 16 or gi > 112) else 3
                    while len(pending) > lag:
                        drain_one()
            while pending:
                drain_one()
            assert fillers_used == n_fillers
            # tail: remaining op units; the very last one splits its
            # evict/store so the final DMA launches earlier
            for qb in range(op_emitted, NQB):
                op_unit(sc_tile, qb)

    nc.compile()
    return nc


_NC_CACHE = {}


def _get_nc(use_mask: bool, fp8proj: bool = True):
    key = (use_mask, fp8proj)
    if key not in _NC_CACHE:
        _NC_CACHE[key] = _build_nc(use_mask, fp8proj)
    return _NC_CACHE[key]


NP_F8 = ml_dtypes.float8_e4m3


def _split8(a):
    """coarse/residual fp8 split: a ~= a8 + ar (elementwise)."""
    a8 = a.astype(NP_F8)
    ar = (a - a8.astype(np.float32)).astype(NP_F8)
    return a8, ar


def _shard_inputs(hidden_states, attention_mask, Wq, bq, Wk, bk, Wv, bv, Wo, bo,
                  use_mask, fp8proj):
    """Build the 8 per-core input maps (all host-side numpy)."""
    in_maps = []
    for c in range(NCORES):
        b, g = divmod(c, NCORES // B)
        cols = slice(g * HPC * HD, (g + 1) * HPC * HD)
        # wqk columns ordered [Q0|Q1|K0|K1|Q2|K2] within the group
        wq_g = Wq[:, cols]
        wk_g = Wk[:, cols]
        qk_cols = [wq_g[:, 0:HD], wq_g[:, HD:2 * HD],
                   wk_g[:, 0:HD], wk_g[:, HD:2 * HD],
                   wq_g[:, 2 * HD:3 * HD], wk_g[:, 2 * HD:3 * HD]]
        wqk = np.concatenate(qk_cols, axis=1)
        m = {"wo": np.ascontiguousarray(Wo[cols, :]).astype(NP_CDT)}
        if fp8proj:
            xt = np.ascontiguousarray(hidden_states[b].T).astype(np.float32)
            x8, xr = _split8(xt)
            w8, wr = _split8(wqk.astype(np.float32) * WSCALE)
            v8, vr = _split8(np.ascontiguousarray(
                Wv[:, cols]).astype(np.float32) * WSCALE)
            m.update(x8=x8, xr=xr, wqk8=w8, wqkr=wr, wv8=v8, wvr=vr)
        else:
            bq_g = bq[cols]
            bk_g = bk[cols]
            bqk = np.concatenate([bq_g[0:HD], bq_g[HD:2 * HD],
                                  bk_g[0:HD], bk_g[HD:2 * HD],
                                  bq_g[2 * HD:3 * HD], bk_g[2 * HD:3 * HD]])
            m.update(
                xt=np.ascontiguousarray(hidden_states[b].T).astype(NP_CDT),
                wqk=np.ascontiguousarray(wqk).astype(NP_CDT),
                wv=np.ascontiguousarray(Wv[:, cols]).astype(NP_CDT),
                bqk=bqk.astype(np.float32).reshape(-1, 1),
            )
        if use_mask:
            mvec = np.exp(-10000.0 * (1.0 - attention_mask[b].astype(np.float64)))
            m["mv"] = mvec.astype(np.float32).reshape(-1, 1)
        in_maps.append(m)
    return in_maps


def kernel(hidden_states, attention_mask, Wq, bq, Wk, bk, Wv, bv, Wo, bo):
    hidden_states = np.asarray(hidden_states, np.float32)
    attention_mask = np.asarray(attention_mask)
    Wq, bq = np.asarray(Wq, np.float32), np.asarray(bq, np.float32)
    Wk, bk = np.asarray(Wk, np.float32), np.asarray(bk, np.float32)
    Wv, bv = np.asarray(Wv, np.float32), np.asarray(bv, np.float32)
    Wo, bo = np.asarray(Wo, np.float32), np.asarray(bo, np.float32)

    use_mask = not bool(np.all(attention_mask == 1))
    # Q/K biases fold into scores on-device; the fp8 projection path is a
    # zero-bias specialization (bv/bo are always handled on the host).
    fp8proj = bool(np.all(bq == 0.0) and np.all(bk == 0.0))
    nc = _get_nc(use_mask, fp8proj)
    in_maps = _shard_inputs(hidden_states, attention_mask,
                            Wq, bq, Wk, bk, Wv, bv, Wo, bo, use_mask, fp8proj)
    res = run_bass_kernel_spmd(nc, in_maps, core_ids=list(range(NCORES)))

    # unshard: sum the 4 head-group partials per batch; add constant row.
    const_row = (bv.astype(np.float64) @ Wo.astype(np.float64)
                 + bo.astype(np.float64))
    out = np.zeros((B, S, H), np.float64)
    for c in range(NCORES):
        b = c // (NCORES // B)
        out[b] += res.results[c]["out"].astype(np.float64)
    if fp8proj:
        out /= WSCALE   # v path carries the global weight scale
    out += const_row[None, None, :]
    return out.astype(np.float32)


if __name__ == "__main__":
    rng = np.random.default_rng(0)
    inputs = {
        "hidden_states": rng.standard_normal((B, S, H)).astype(np.float32),
        "attention_mask": np.ones((B, S), np.int32),
        "Wq": rng.standard_normal((H, H)).astype(np.float32) * 0.02,
        "bq": np.zeros(H, np.float32),
        "Wk": rng.standard_normal((H, H)).astype(np.float32) * 0.02,
        "bk": np.zeros(H, np.float32),
        "Wv": rng.standard_normal((H, H)).astype(np.float32) * 0.02,
        "bv": np.zeros(H, np.float32),
        "Wo": rng.standard_normal((H, H)).astype(np.float32) * 0.02,
        "bo": np.zeros(H, np.float32),
    }
    out = kernel(**inputs)
    print("out", out.shape, out.dtype)
